# revision 3
# baseline (speedup 1.0000x reference)
"""Block-quantize kernel for Trainium2 (8 NeuronCores, data-parallel).

Reference semantics (fp32, wl=8, ebit=8):
    m  = max(max|x|, 1e-10)                      # global over all elements
    e  = clip(floor(log2(m)), -128, 127)
    y  = clip(round_half_even(x * 2^(6-e)), -128, 127) * 2^(e-6)

Implementation:
  - x (16, 2048, 4096) f32 is sharded on the batch dim: 2 batches per core,
    viewed per-core as (2048, 8192) so each [128, 8192] tile is one
    contiguous 4 MiB DMA.
  - Pass 1 streams the shard computing abs-max (DVE tensor_reduce with
    apply_absolute_value), reduces across partitions (GpSimd
    partition_all_reduce), then a 4-byte AllReduce(max) across the 8 cores.
  - e and the two power-of-two scales are derived with exact int32 bit
    arithmetic on the fp32 representation (all values are multiples of 2^23
    with small multipliers, so the DVE's internal fp32 math is exact):
        p  = bits(m) & 0x7F800000            # bits of 2^e
        s2 = bits^-1(p - (6<<23))            # 2^(e-6)
        s1 = bits^-1(((254<<23) - p) + (6<<23))   # 2^(6-e)
  - Pass 2 streams the shard again:
        r  = x*s1 + C        # C = 1.5*2^23; fp32 RNE add == round-half-even
        u  = min(r, C+127) ; max(u, C-128)        # clip in shifted domain
        y  = (u - C) * s2                         # both steps exact in fp32
    Every elementwise op is a dual-op DVE tensor_scalar (2x perf mode).
"""
import sys

if "/opt/trn_rl_repo" not in sys.path:
    sys.path.insert(0, "/opt/trn_rl_repo")

import numpy as np

N_CORES = 8
B, S, D = 16, 2048, 4096          # full input shape
PB = B // N_CORES                  # batches per core
P = 128                            # SBUF partitions
F = 8192                           # tile free dim  -> [128, 8192] = 4 MiB
ROWS = PB * S * D // F             # 2048 per-core rows of 8192
T = ROWS // P                      # 16 tiles per core
C_MAGIC = 12582912.0               # 1.5 * 2^23, round-to-nearest-even magic

_CACHE = {}


def _build(reps: int = 1):
    import concourse.mybir as mybir
    from concourse import bacc, bass_isa, tile

    DT = mybir.dt.float32
    DI = mybir.dt.int32
    A = mybir.AluOpType

    nc = bacc.Bacc("TRN2", target_bir_lowering=False, debug=False,
                   num_devices=N_CORES)
    x = nc.dram_tensor("x", [ROWS, F], DT, kind="ExternalInput")
    y = nc.dram_tensor("y", [ROWS, F], DT, kind="ExternalOutput")

    with tile.TileContext(nc) as tc:
        with tc.tile_pool(name="data", bufs=5) as data, \
             tc.tile_pool(name="small", bufs=reps) as small, \
             tc.tile_pool(name="dram", bufs=1, space="DRAM") as dram:
          for _rep in range(reps):
            # ---------------- pass 1: local abs-max ----------------
            stats = small.tile([P, T], DT, tag="stats")
            for i in range(T):
                t = data.tile([P, F], DT, tag="blk")
                nc.sync.dma_start(out=t[:], in_=x[i * P:(i + 1) * P, :])
                nc.vector.tensor_reduce(out=stats[:, i:i + 1], in_=t[:],
                                        axis=mybir.AxisListType.X,
                                        op=A.max, apply_absolute_value=True)
            lmax = small.tile([P, 1], DT, tag="lmax")
            nc.vector.tensor_reduce(out=lmax[:], in_=stats[:],
                                    axis=mybir.AxisListType.X, op=A.max)
            amax = small.tile([P, 1], DT, tag="amax")
            nc.gpsimd.partition_all_reduce(amax[:], lmax[:], channels=P,
                                           reduce_op=bass_isa.ReduceOp.max)
            # zeros map to 1e-10 in the reference, so m >= 1e-10
            nc.vector.tensor_scalar(out=amax[:], in0=amax[:], scalar1=1e-10,
                                    scalar2=None, op0=A.max)

            # -------- all-reduce(max) of one scalar across 8 cores --------
            cc_in = dram.tile([1, 1], DT, tag="cc_in")
            cc_out = dram.tile([1, 1], DT, tag="cc_out")
            nc.sync.dma_start(out=cc_in[:], in_=amax[0:1, 0:1])
            nc.gpsimd.collective_compute(
                "AllReduce", A.max,
                replica_groups=[list(range(N_CORES))],
                ins=[cc_in[:]], outs=[cc_out[:]],
            )
            gm1 = small.tile([1, 1], DT, tag="gm1")
            nc.sync.dma_start(out=gm1[:], in_=cc_out[:])
            gmax = small.tile([P, 1], DT, tag="gmax")
            nc.gpsimd.partition_broadcast(gmax[:], gm1[:])

            # ------------- scales via exact bit arithmetic -------------
            bits = gmax[:].bitcast(DI)
            p_i = small.tile([P, 1], DI, tag="p_i")
            nc.vector.tensor_scalar(out=p_i[:], in0=bits, scalar1=0x7F800000,
                                    scalar2=None, op0=A.bitwise_and)
            s2i = small.tile([P, 1], DI, tag="s2i")
            nc.vector.tensor_scalar(out=s2i[:], in0=p_i[:], scalar1=6 << 23,
                                    scalar2=None, op0=A.subtract)
            s1i = small.tile([P, 1], DI, tag="s1i")
            nc.vector.tensor_scalar(out=s1i[:], in0=p_i[:], scalar1=254 << 23,
                                    scalar2=-1.0, op0=A.subtract, op1=A.mult)
            nc.vector.tensor_scalar(out=s1i[:], in0=s1i[:], scalar1=6 << 23,
                                    scalar2=None, op0=A.add)
            s1 = s1i[:].bitcast(DT)
            s2 = s2i[:].bitcast(DT)

            # ---------------- pass 2: quantize ----------------
            for i in range(T):
                t = data.tile([P, F], DT, tag="blk")
                nc.sync.dma_start(out=t[:], in_=x[i * P:(i + 1) * P, :])
                nc.vector.tensor_scalar(out=t[:], in0=t[:], scalar1=s1,
                                        scalar2=C_MAGIC,
                                        op0=A.mult, op1=A.add)
                nc.vector.tensor_scalar(out=t[:], in0=t[:],
                                        scalar1=C_MAGIC + 127.0,
                                        scalar2=C_MAGIC - 128.0,
                                        op0=A.min, op1=A.max)
                nc.vector.tensor_scalar(out=t[:], in0=t[:], scalar1=-C_MAGIC,
                                        scalar2=s2, op0=A.add, op1=A.mult)
                nc.sync.dma_start(out=y[i * P:(i + 1) * P, :], in_=t[:])

    nc.compile()
    return nc


def _get_nc(reps: int = 1):
    if reps not in _CACHE:
        _CACHE[reps] = _build(reps)
    return _CACHE[reps]


def kernel(x: np.ndarray) -> np.ndarray:
    from concourse.bass_utils import run_bass_kernel_spmd

    x = np.ascontiguousarray(x, dtype=np.float32)
    assert x.shape == (B, S, D), x.shape
    nc = _get_nc()
    in_maps = [
        {"x": x[c * PB:(c + 1) * PB].reshape(ROWS, F)} for c in range(N_CORES)
    ]
    res = run_bass_kernel_spmd(nc, in_maps, core_ids=list(range(N_CORES)))
    out = np.empty((B, S, D), dtype=np.float32)
    for c in range(N_CORES):
        out[c * PB:(c + 1) * PB] = res.results[c]["y"].reshape(PB, S, D)
    return out


# revision 8
# speedup vs baseline: 11.4059x; 11.4059x over previous
"""Block-quantize kernel for Trainium2 (8 NeuronCores, data-parallel).

Reference semantics (fp32, wl=8, ebit=8):
    m  = max(max|x|, 1e-10)                      # global over all elements
    e  = clip(floor(log2(m)), -128, 127)
    y  = clip(round_half_even(x * 2^(6-e)), -128, 127) * 2^(e-6)

Implementation:
  - x (16, 2048, 4096) f32 is sharded on the batch dim: 2 batches per core
    (64 MiB), treated as a flat per-core vector so every [128, TILE_F] tile
    is one contiguous DMA.
  - Pass 1 streams the shard computing abs-max (DVE tensor_reduce with
    apply_absolute_value), reduces across partitions (GpSimd
    partition_all_reduce), then a 4-byte AllReduce(max) across the 8 cores.
  - e and the two power-of-two scales are derived with exact int32 bit
    arithmetic on the fp32 representation (all values are multiples of 2^23
    with small multipliers, so the DVE's internal fp32 math is exact):
        p  = bits(m) & 0x7F800000                 # bits of 2^e
        s2 = bits^-1(p - (6<<23))                 # 2^(e-6)
        s1 = bits^-1(((254<<23) - p) + (6<<23))   # 2^(6-e)
  - Pass 2 streams the shard again:
        r  = x*s1 + C        # C = 1.5*2^23; fp32 RNE add == round-half-even
        u  = min(r, C+127) ; max(u, C-128)        # clip in shifted domain
        y  = (u - C) * s2                         # both steps exact in fp32
    Every elementwise op is a dual-op DVE tensor_scalar (2x perf mode).
  - All four elementwise/reduce values stay exact in fp32, so the result is
    bit-identical to the reference evaluated in exact arithmetic.
"""
import sys

if "/opt/trn_rl_repo" not in sys.path:
    sys.path.insert(0, "/opt/trn_rl_repo")

import numpy as np

N_CORES = 8
B, S, D = 16, 2048, 4096          # full input shape
PB = B // N_CORES                  # batches per core
P = 128                            # SBUF partitions
NELEM = PB * S * D                 # per-core elements (16.8M, 64 MiB)
TILE_F = 4096                      # tile free dim -> [128, 4096] = 2 MiB
BUFS = 10                          # data-pool slots
C_MAGIC = 12582912.0               # 1.5 * 2^23, round-to-nearest-even magic

_CACHE = {}


def _build(reps: int = 1, tile_f: int = TILE_F, bufs: int = BUFS):
    import concourse.mybir as mybir
    from concourse import bacc, bass_isa, tile

    DT = mybir.dt.float32
    DI = mybir.dt.int32
    A = mybir.AluOpType

    ch = P * tile_f                # elements per tile
    n_t = NELEM // ch              # tiles per pass
    assert n_t * ch == NELEM

    nc = bacc.Bacc("TRN2", target_bir_lowering=False, debug=False,
                   num_devices=N_CORES)
    x = nc.dram_tensor("x", [NELEM], DT, kind="ExternalInput")
    y = nc.dram_tensor("y", [NELEM], DT, kind="ExternalOutput")

    def blk(dram, i):
        return dram[i * ch:(i + 1) * ch].rearrange("(p f) -> p f", f=tile_f)

    with tile.TileContext(nc) as tc:
        with tc.tile_pool(name="data", bufs=bufs) as data, \
             tc.tile_pool(name="small", bufs=reps) as small, \
             tc.tile_pool(name="dram", bufs=1, space="DRAM") as dram:
          for _rep in range(reps):
            # ---------------- pass 1: local abs-max ----------------
            stats = small.tile([P, n_t], DT, tag="stats")
            for i in range(n_t):
                t = data.tile([P, tile_f], DT, tag="blk")
                nc.sync.dma_start(out=t[:], in_=blk(x, i))
                nc.vector.tensor_reduce(out=stats[:, i:i + 1], in_=t[:],
                                        axis=mybir.AxisListType.X,
                                        op=A.max, apply_absolute_value=True)
            lmax = small.tile([P, 1], DT, tag="lmax")
            nc.vector.tensor_reduce(out=lmax[:], in_=stats[:],
                                    axis=mybir.AxisListType.X, op=A.max)
            amax = small.tile([P, 1], DT, tag="amax")
            nc.gpsimd.partition_all_reduce(amax[:], lmax[:], channels=P,
                                           reduce_op=bass_isa.ReduceOp.max)
            # zeros map to 1e-10 in the reference, so m >= 1e-10
            nc.vector.tensor_scalar(out=amax[:], in0=amax[:], scalar1=1e-10,
                                    scalar2=None, op0=A.max)

            # -------- all-reduce(max) of one scalar across 8 cores --------
            cc_in = dram.tile([1, 1], DT, tag="cc_in")
            cc_out = dram.tile([1, 1], DT, tag="cc_out")
            nc.sync.dma_start(out=cc_in[:], in_=amax[0:1, 0:1])
            nc.gpsimd.collective_compute(
                "AllReduce", A.max,
                replica_groups=[list(range(N_CORES))],
                ins=[cc_in[:]], outs=[cc_out[:]],
            )
            gm1 = small.tile([1, 1], DT, tag="gm1")
            nc.sync.dma_start(out=gm1[:], in_=cc_out[:])
            gmax = small.tile([P, 1], DT, tag="gmax")
            nc.gpsimd.partition_broadcast(gmax[:], gm1[:])

            # ------------- scales via exact bit arithmetic -------------
            bits = gmax[:].bitcast(DI)
            p_i = small.tile([P, 1], DI, tag="p_i")
            nc.vector.tensor_scalar(out=p_i[:], in0=bits, scalar1=0x7F800000,
                                    scalar2=None, op0=A.bitwise_and)
            s2i = small.tile([P, 1], DI, tag="s2i")
            nc.vector.tensor_scalar(out=s2i[:], in0=p_i[:], scalar1=6 << 23,
                                    scalar2=None, op0=A.subtract)
            s1i = small.tile([P, 1], DI, tag="s1i")
            nc.vector.tensor_scalar(out=s1i[:], in0=p_i[:], scalar1=254 << 23,
                                    scalar2=-1.0, op0=A.subtract, op1=A.mult)
            nc.vector.tensor_scalar(out=s1i[:], in0=s1i[:], scalar1=6 << 23,
                                    scalar2=None, op0=A.add)
            s1 = s1i[:].bitcast(DT)
            s2 = s2i[:].bitcast(DT)

            # ---------------- pass 2: quantize ----------------
            for i in range(n_t):
                t = data.tile([P, tile_f], DT, tag="blk")
                nc.sync.dma_start(out=t[:], in_=blk(x, i))
                nc.vector.tensor_scalar(out=t[:], in0=t[:], scalar1=s1,
                                        scalar2=C_MAGIC,
                                        op0=A.mult, op1=A.add)
                nc.vector.tensor_scalar(out=t[:], in0=t[:],
                                        scalar1=C_MAGIC + 127.0,
                                        scalar2=C_MAGIC - 128.0,
                                        op0=A.min, op1=A.max)
                nc.vector.tensor_scalar(out=t[:], in0=t[:], scalar1=-C_MAGIC,
                                        scalar2=s2, op0=A.add, op1=A.mult)
                nc.sync.dma_start(out=blk(y, i), in_=t[:])

    nc.compile()
    return nc


def _get_nc(reps: int = 1, tile_f: int = TILE_F, bufs: int = BUFS):
    key = (reps, tile_f, bufs)
    if key not in _CACHE:
        _CACHE[key] = _build(reps, tile_f, bufs)
    return _CACHE[key]


def kernel(x: np.ndarray) -> np.ndarray:
    from concourse.bass_utils import run_bass_kernel_spmd

    x = np.ascontiguousarray(np.asarray(x), dtype=np.float32)
    assert x.shape == (B, S, D), x.shape
    nc = _get_nc()
    flat = x.reshape(N_CORES, NELEM)
    in_maps = [{"x": flat[c]} for c in range(N_CORES)]
    res = run_bass_kernel_spmd(nc, in_maps, core_ids=list(range(N_CORES)))
    out = np.empty((B, S, D), dtype=np.float32)
    for c in range(N_CORES):
        out[c * PB:(c + 1) * PB] = res.results[c]["y"].reshape(PB, S, D)
    return out


# revision 10
# speedup vs baseline: 13.3187x; 1.1677x over previous
"""Block-quantize kernel for Trainium2 (8 NeuronCores, data-parallel).

Reference semantics (fp32, wl=8, ebit=8):
    m  = max(max|x|, 1e-10)                      # global over all elements
    e  = clip(floor(log2(m)), -128, 127)
    y  = clip(round_half_even(x * 2^(6-e)), -128, 127) * 2^(e-6)

Implementation:
  - x (16, 2048, 4096) f32 is sharded on the batch dim: 2 batches per core
    (64 MiB), treated as a flat per-core vector so every [128, TILE_F] tile
    is one contiguous DMA.
  - Pass 1 streams the shard computing abs-max (DVE tensor_reduce with
    apply_absolute_value), reduces across partitions (GpSimd
    partition_all_reduce), then a 4-byte AllReduce(max) across the 8 cores.
  - e and the two power-of-two scales are derived with exact int32 bit
    arithmetic on the fp32 representation (all values are multiples of 2^23
    with small multipliers, so the DVE's internal fp32 math is exact):
        p  = bits(m) & 0x7F800000                 # bits of 2^e
        s2 = bits^-1(p - (6<<23))                 # 2^(e-6)
        s1 = bits^-1(((254<<23) - p) + (6<<23))   # 2^(6-e)
  - The last KEEP pass-1 tiles stay resident in SBUF (SBUF fits 12 of the
    32 tiles), so pass 2 skips re-reading 18 MiB of the 64 MiB shard:
    total HBM traffic 174 MiB/core vs the naive two-pass 192 MiB.
  - Pass 2 streams the rest of the shard again:
        r  = x*s1 + C        # C = 1.5*2^23; fp32 RNE add == round-half-even
        u  = min(r, C+127) ; max(u, C-128)        # clip in shifted domain
        y  = (u - C) * s2                         # both steps exact in fp32
    Every elementwise op is a dual-op DVE tensor_scalar (2x perf mode).
  - All four elementwise/reduce values stay exact in fp32, so the result is
    bit-identical to the reference evaluated in exact arithmetic.
"""
import sys

if "/opt/trn_rl_repo" not in sys.path:
    sys.path.insert(0, "/opt/trn_rl_repo")

import numpy as np

N_CORES = 8
B, S, D = 16, 2048, 4096          # full input shape
PB = B // N_CORES                  # batches per core
P = 128                            # SBUF partitions
NELEM = PB * S * D                 # per-core elements (16.8M, 64 MiB)
TILE_F = 4096                      # tile free dim -> [128, 4096] = 2 MiB
BUFS = 3                           # streaming-pool slots
KEEP = 9                           # pass-1 tail tiles kept in SBUF for pass 2
C_MAGIC = 12582912.0               # 1.5 * 2^23, round-to-nearest-even magic

_CACHE = {}


def _build(reps: int = 1, tile_f: int = TILE_F, bufs: int = BUFS,
           clip_engine: str = "vector", keep: int = KEEP):
    import concourse.mybir as mybir
    from concourse import bacc, bass_isa, tile

    DT = mybir.dt.float32
    DI = mybir.dt.int32
    A = mybir.AluOpType

    ch = P * tile_f                # elements per tile
    n_t = NELEM // ch              # tiles per pass
    assert n_t * ch == NELEM
    n_keep = min(keep, n_t - 1)    # tail tiles that stay resident in SBUF
    n_stream = n_t - n_keep

    nc = bacc.Bacc("TRN2", target_bir_lowering=False, debug=False,
                   num_devices=N_CORES)
    x = nc.dram_tensor("x", [NELEM], DT, kind="ExternalInput")
    y = nc.dram_tensor("y", [NELEM], DT, kind="ExternalOutput")

    def blk(dram, i):
        return dram[i * ch:(i + 1) * ch].rearrange("(p f) -> p f", f=tile_f)

    with tile.TileContext(nc) as tc:
        with tc.tile_pool(name="data", bufs=bufs) as data, \
             tc.tile_pool(name="keep", bufs=max(n_keep, 1)) as keepp, \
             tc.tile_pool(name="small", bufs=reps) as small, \
             tc.tile_pool(name="dram", bufs=1, space="DRAM") as dram:
          for _rep in range(reps):
            # ---------------- pass 1: local abs-max ----------------
            # the last n_keep tiles load into a dedicated pool and stay
            # resident so pass 2 skips re-reading them from HBM
            stats = small.tile([P, n_t], DT, tag="stats")
            kept = []
            for i in range(n_t):
                if i < n_stream:
                    t = data.tile([P, tile_f], DT, tag="blk")
                else:
                    t = keepp.tile([P, tile_f], DT, tag="keep")
                    kept.append(t)
                nc.sync.dma_start(out=t[:], in_=blk(x, i))
                nc.vector.tensor_reduce(out=stats[:, i:i + 1], in_=t[:],
                                        axis=mybir.AxisListType.X,
                                        op=A.max, apply_absolute_value=True)
            lmax = small.tile([P, 1], DT, tag="lmax")
            nc.vector.tensor_reduce(out=lmax[:], in_=stats[:],
                                    axis=mybir.AxisListType.X, op=A.max)
            amax = small.tile([P, 1], DT, tag="amax")
            nc.gpsimd.partition_all_reduce(amax[:], lmax[:], channels=P,
                                           reduce_op=bass_isa.ReduceOp.max)
            # zeros map to 1e-10 in the reference, so m >= 1e-10
            nc.vector.tensor_scalar(out=amax[:], in0=amax[:], scalar1=1e-10,
                                    scalar2=None, op0=A.max)

            # -------- all-reduce(max) of one scalar across 8 cores --------
            cc_in = dram.tile([1, 1], DT, tag="cc_in")
            cc_out = dram.tile([1, 1], DT, tag="cc_out")
            nc.sync.dma_start(out=cc_in[:], in_=amax[0:1, 0:1])
            nc.gpsimd.collective_compute(
                "AllReduce", A.max,
                replica_groups=[list(range(N_CORES))],
                ins=[cc_in[:]], outs=[cc_out[:]],
            )
            gm1 = small.tile([1, 1], DT, tag="gm1")
            nc.sync.dma_start(out=gm1[:], in_=cc_out[:])
            gmax = small.tile([P, 1], DT, tag="gmax")
            nc.gpsimd.partition_broadcast(gmax[:], gm1[:])

            # ------------- scales via exact bit arithmetic -------------
            bits = gmax[:].bitcast(DI)
            p_i = small.tile([P, 1], DI, tag="p_i")
            nc.vector.tensor_scalar(out=p_i[:], in0=bits, scalar1=0x7F800000,
                                    scalar2=None, op0=A.bitwise_and)
            s2i = small.tile([P, 1], DI, tag="s2i")
            nc.vector.tensor_scalar(out=s2i[:], in0=p_i[:], scalar1=6 << 23,
                                    scalar2=None, op0=A.subtract)
            s1i = small.tile([P, 1], DI, tag="s1i")
            nc.vector.tensor_scalar(out=s1i[:], in0=p_i[:], scalar1=254 << 23,
                                    scalar2=-1.0, op0=A.subtract, op1=A.mult)
            nc.vector.tensor_scalar(out=s1i[:], in0=s1i[:], scalar1=6 << 23,
                                    scalar2=None, op0=A.add)
            s1 = s1i[:].bitcast(DT)
            s2 = s2i[:].bitcast(DT)

            # ---------------- pass 2: quantize ----------------
            # kept tiles first: DVE has work immediately after the
            # collective while the streaming loads ramp back up
            def quantize(t):
                nc.vector.tensor_scalar(out=t[:], in0=t[:], scalar1=s1,
                                        scalar2=C_MAGIC,
                                        op0=A.mult, op1=A.add)
                eng = getattr(nc, clip_engine)
                eng.tensor_scalar(out=t[:], in0=t[:],
                                  scalar1=C_MAGIC + 127.0,
                                  scalar2=C_MAGIC - 128.0,
                                  op0=A.min, op1=A.max)
                nc.vector.tensor_scalar(out=t[:], in0=t[:], scalar1=-C_MAGIC,
                                        scalar2=s2, op0=A.add, op1=A.mult)

            for j, t in enumerate(kept):
                quantize(t)
                nc.sync.dma_start(out=blk(y, n_stream + j), in_=t[:])
            for i in range(n_stream):
                t = data.tile([P, tile_f], DT, tag="blk")
                nc.sync.dma_start(out=t[:], in_=blk(x, i))
                quantize(t)
                nc.sync.dma_start(out=blk(y, i), in_=t[:])

    nc.compile()
    return nc


def _get_nc(reps: int = 1, tile_f: int = TILE_F, bufs: int = BUFS,
            clip_engine: str = "vector", keep: int = KEEP):
    key = (reps, tile_f, bufs, clip_engine, keep)
    if key not in _CACHE:
        _CACHE[key] = _build(reps, tile_f, bufs, clip_engine, keep)
    return _CACHE[key]


def kernel(x: np.ndarray) -> np.ndarray:
    from concourse.bass_utils import run_bass_kernel_spmd

    x = np.ascontiguousarray(np.asarray(x), dtype=np.float32)
    assert x.shape == (B, S, D), x.shape
    nc = _get_nc()
    flat = x.reshape(N_CORES, NELEM)
    in_maps = [{"x": flat[c]} for c in range(N_CORES)]
    res = run_bass_kernel_spmd(nc, in_maps, core_ids=list(range(N_CORES)))
    out = np.empty((B, S, D), dtype=np.float32)
    for c in range(N_CORES):
        out[c * PB:(c + 1) * PB] = res.results[c]["y"].reshape(PB, S, D)
    return out
